# revision 6
# baseline (speedup 1.0000x reference)
"""Multi-head attention Bass/Tile kernel for TRN2, 8-core SPMD.

Sharding: core c handles batch b = c//2 and head-group g = c%2 (6 of 12 heads).
Each core computes its 6 heads end-to-end plus a partial output projection
(over its 384 of 768 ctx dims); the host sums the two partials per batch.

v2 design (vs baseline): all matmuls bf16 (PSUM accumulation stays f32),
ScalarE exp stream (192 x [128,1024] = ~197us) is the floor; all other
work (QKV projections, scores, ctx, out-proj) is interleaved under it in
one dense PE instruction queue to keep the tensor engine at full p-state:
  lead-in:  m0/m1 q/k chains + first 6 v tiles (DMA-ordered hs-first)
  phase 2:  per (s-block, head-pair): score pair -> exp -> ctx pend-by-1,
            one background unit (v tile / m2 chain / out-proj chain) per step
  normalize: Z row collected via DMA from the ctx PSUM evacuation tile,
            one [6,512] reciprocal per s-block (not 24 x [1,512]),
            gpsimd partition-broadcast + vector multiply
  out-proj: fused per s-block into the next block's background slots
"""

from collections import deque
from contextlib import ExitStack

import numpy as np
import ml_dtypes

import concourse.bass as bass
import concourse.tile as tile
from concourse import bacc, mybir
from concourse._compat import with_exitstack

F32 = mybir.dt.float32
BF16 = mybir.dt.bfloat16
AF = mybir.ActivationFunctionType

B, E, S, H, D = 4, 768, 2048, 12, 64
NH = 6          # heads per core
HD = NH * D     # 384 head-dims per core
NE = E // 128   # 6 e-chunks
NM = HD // 128  # 3 m-chunks (2 heads each)
NT = S // 128   # 16 t-tiles
SBW = 512       # s-block width
NS = S // SBW   # 4 s-blocks
VW = 96         # ctx stationary width: col 0 = ones (denominator), 32:96 = v


@with_exitstack
def mha_tile(ctx: ExitStack, tc, hs, wq, wk, wv, bq, bk, bv, woT, bo2, outT):
    nc = tc.nc

    persist = ctx.enter_context(tc.tile_pool(name="persist", bufs=1))

    # --- persistent SBUF tiles ---
    hs_sb = [persist.tile([128, S], BF16, name=f"hs{e}") for e in range(NE)]
    wq_sb = [persist.tile([128, HD], BF16, name=f"wq{e}") for e in range(NE)]
    wk_sb = [persist.tile([128, HD], BF16, name=f"wk{e}") for e in range(NE)]
    wv_sb = [persist.tile([128, HD], BF16, name=f"wv{e}") for e in range(NE)]
    woT_sb = [persist.tile([128, E], BF16, name=f"wo{f}") for f in range(NM)]
    qT_sb = [persist.tile([128, S], BF16, name=f"qT{m}") for m in range(NM)]
    kT_sb = [persist.tile([128, S], BF16, name=f"kT{m}") for m in range(NM)]
    ctxT_sb = [persist.tile([128, S], BF16, name=f"ctxT{m}") for m in range(NM)]
    v_aug = [persist.tile([128, NH, VW], BF16, name=f"vaug{t}") for t in range(NT)]
    zr = persist.tile([NH, S], F32, name="zr")      # denominators
    zrec = persist.tile([NH, S], F32, name="zrec")  # reciprocals

    bq_sb = persist.tile([128, NM], F32, name="bq")
    bk_sb = persist.tile([128, NM], F32, name="bk")
    bv_bc = persist.tile([128, HD], F32, name="bv")
    bo_sb = persist.tile([128, NE], F32, name="bo")

    # --- DMA issue order: hs+wq+wk first (gates lead-in), rest after ---
    for e in range(NE):
        sl = slice(128 * e, 128 * (e + 1))
        nc.sync.dma_start(hs_sb[e][:], hs[sl, :])
        nc.sync.dma_start(wq_sb[e][:], wq[sl, :])
        nc.sync.dma_start(wk_sb[e][:], wk[sl, :])
    for e in range(NE):
        nc.sync.dma_start(wv_sb[e][:], wv[128 * e : 128 * (e + 1), :])
    for f in range(NM):
        nc.sync.dma_start(woT_sb[f][:], woT[128 * f : 128 * (f + 1), :])
    nc.sync.dma_start(bq_sb[:], bq.rearrange("(m p) -> p m", p=128))
    nc.sync.dma_start(bk_sb[:], bk.rearrange("(m p) -> p m", p=128))
    nc.sync.dma_start(
        bv_bc[:], bass.AP(tensor=bv.tensor, offset=bv.offset, ap=[[0, 128], [1, HD]])
    )
    nc.sync.dma_start(bo_sb[:], bo2.rearrange("(m p) -> p m", p=128))

    # v_aug init on gpsimd (vector stays free): zeros, then ones in col 0
    for t in range(NT):
        nc.gpsimd.memset(v_aug[t][:].rearrange("p h d -> p (h d)"), 0.0)
        nc.gpsimd.memset(v_aug[t][:, :, 0:1], 1.0)

    # --- PSUM pools (8 banks total: 4 sc + 2 ctx + 2 misc) ---
    pssc = ctx.enter_context(tc.tile_pool(name="pssc", bufs=2, space="PSUM"))
    psctx = ctx.enter_context(tc.tile_pool(name="psctx", bufs=1, space="PSUM"))
    psmisc = ctx.enter_context(tc.tile_pool(name="psmisc", bufs=2, space="PSUM"))

    # SBUF working pools
    expp = ctx.enter_context(tc.tile_pool(name="expp", bufs=4))
    cup = ctx.enter_context(tc.tile_pool(name="cup", bufs=6))
    bcp = ctx.enter_context(tc.tile_pool(name="bcp", bufs=6))
    zhp = ctx.enter_context(tc.tile_pool(name="zhp", bufs=6))
    outp = ctx.enter_context(tc.tile_pool(name="outp", bufs=2))

    # --- work units (each: one PE chain + its PSUM evacuation) ---
    def q_unit(m, s):
        msl = slice(128 * m, 128 * (m + 1))
        ssl = slice(SBW * s, SBW * (s + 1))
        qp = psmisc.tile([128, SBW], F32, tag="misc")
        for e in range(NE):
            nc.tensor.matmul(
                qp[:], wq_sb[e][:, msl], hs_sb[e][:, ssl],
                start=(e == 0), stop=(e == NE - 1),
            )
        nc.vector.tensor_scalar_add(
            out=qT_sb[m][:, ssl], in0=qp[:], scalar1=bq_sb[:, m : m + 1]
        )

    def k_unit(m, s):
        msl = slice(128 * m, 128 * (m + 1))
        ssl = slice(SBW * s, SBW * (s + 1))
        kp = psmisc.tile([128, SBW], F32, tag="misc")
        for e in range(NE):
            nc.tensor.matmul(
                kp[:], wk_sb[e][:, msl], hs_sb[e][:, ssl],
                start=(e == 0), stop=(e == NE - 1),
            )
        nc.vector.tensor_scalar_add(
            out=kT_sb[m][:, ssl], in0=kp[:], scalar1=bk_sb[:, m : m + 1]
        )

    def v_unit(t):
        tsl = slice(128 * t, 128 * (t + 1))
        vp = psmisc.tile([128, SBW], F32, tag="misc")
        for e in range(NE):
            nc.tensor.matmul(
                vp[:, 0:HD], hs_sb[e][:, tsl], wv_sb[e][:, :],
                start=(e == 0), stop=(e == NE - 1),
            )
        nc.vector.tensor_add(
            out=v_aug[t][:, :, 32 : 32 + D],
            in0=vp[:, 0:HD].rearrange("p (h d) -> p h d", h=NH),
            in1=bv_bc[:].rearrange("p (h d) -> p h d", h=NH),
        )

    def out_unit(s, et):
        esl = slice(128 * et, 128 * (et + 1))
        ssl = slice(SBW * s, SBW * (s + 1))
        op = psmisc.tile([128, SBW], F32, tag="misc")
        for f in range(NM):
            nc.tensor.matmul(
                op[:], woT_sb[f][:, esl], ctxT_sb[f][:, ssl],
                start=(f == 0), stop=(f == NM - 1),
            )
        ob = outp.tile([128, SBW], F32, tag="ob")
        nc.vector.tensor_scalar_add(
            out=ob[:], in0=op[:], scalar1=bo_sb[:, et : et + 1]
        )
        nc.sync.dma_start(outT[esl, ssl], ob[:])

    # --- lead-in: m0 + m1 q/k and first v tiles ---
    for s in range(NS):
        q_unit(0, s)
        k_unit(0, s)
    for s in range(NS):
        q_unit(1, s)
        k_unit(1, s)
    for t in range(6):
        v_unit(t)

    bg = deque()
    for t in range(6, NT):
        bg.append(lambda t=t: v_unit(t))
    for s in range(NS):
        bg.append(lambda s=s: q_unit(2, s))
        bg.append(lambda s=s: k_unit(2, s))

    # --- phase 2: attention, one [128,1024] exp per (pair, t-tile) covering
    # both heads; ctx pend-by-1; one background unit per step ---
    for s in range(NS):
        ssl = slice(SBW * s, SBW * (s + 1))
        cus = []
        for p in range(NM):
            kTh = kT_sb[p]
            qTh = qT_sb[p]
            cpA = psctx.tile([128, SBW], F32, tag="ctxA")
            cpB = psctx.tile([128, SBW], F32, tag="ctxB")
            pend = None

            def ctx_mms(ex, t, stop):
                st = t == 0
                nc.tensor.matmul(
                    cpA[0:VW, :], v_aug[t][:, 2 * p, :], ex[:, 0:SBW],
                    start=st, stop=stop,
                )
                nc.tensor.matmul(
                    cpB[0:VW, :], v_aug[t][:, 2 * p + 1, :], ex[:, SBW : 2 * SBW],
                    start=st, stop=stop,
                )

            for t in range(NT):
                tsl = slice(128 * t, 128 * (t + 1))
                sc = pssc.tile([128, 2 * SBW], F32, tag="sc")
                nc.tensor.matmul(
                    sc[:, 0:SBW], kTh[0:D, tsl], qTh[0:D, ssl],
                    start=True, stop=True,
                )
                nc.tensor.matmul(
                    sc[:, SBW : 2 * SBW], kTh[D:128, tsl], qTh[D:128, ssl],
                    start=True, stop=True,
                )
                ex = expp.tile([128, 2 * SBW], BF16, tag="exp")
                nc.scalar.activation(ex[:], sc[:], AF.Exp)
                if pend is not None:
                    ctx_mms(pend[0], pend[1], stop=False)
                pend = (ex, t)
                if bg:
                    bg.popleft()()
            ctx_mms(pend[0], pend[1], stop=True)

            # evacuate ctx PSUM (rows 0:96: Z row + 64 v rows), collect Z
            for h, cp in ((2 * p, cpA), (2 * p + 1, cpB)):
                cu = cup.tile([VW, SBW], F32, tag="cu")
                nc.vector.tensor_copy(cu[:], cp[0:VW, :])
                nc.sync.dma_start(zr[h : h + 1, ssl], cu[0:1, :])
                cus.append(cu)

        # one reciprocal for all 6 heads of this s-block; per-head rows are
        # DMA'd to partition-0 tiles (engines can't read at partition h)
        nc.vector.reciprocal(out=zrec[:, ssl], in_=zr[:, ssl])
        for p in range(NM):
            for a in range(2):
                h = 2 * p + a
                zh = zhp.tile([1, SBW], F32, tag="zh")
                nc.sync.dma_start(zh[0:1, :], zrec[h : h + 1, ssl])
                bcs = bcp.tile([VW, SBW], F32, tag="bcs")
                nc.gpsimd.partition_broadcast(bcs[:], zh[0:1, :])
                for q in range(2):  # 32-partition chunks (alignment rules)
                    nc.vector.tensor_mul(
                        out=ctxT_sb[p][D * a + 32 * q : D * a + 32 * (q + 1), ssl],
                        in0=cus[2 * p + a][32 + 32 * q : 64 + 32 * q, :],
                        in1=bcs[32 + 32 * q : 64 + 32 * q, :],
                    )
        for et in range(NE):
            bg.append(lambda s=s, et=et: out_unit(s, et))

    # --- drain remaining background (last s-block's out-proj) ---
    while bg:
        bg.popleft()()


def build_nc():
    nc = bacc.Bacc("TRN2", target_bir_lowering=False, debug=False)
    hs = nc.dram_tensor("hs", [E, S], BF16, kind="ExternalInput")
    wq = nc.dram_tensor("wq", [E, HD], BF16, kind="ExternalInput")
    wk = nc.dram_tensor("wk", [E, HD], BF16, kind="ExternalInput")
    wv = nc.dram_tensor("wv", [E, HD], BF16, kind="ExternalInput")
    bq = nc.dram_tensor("bq", [HD], F32, kind="ExternalInput")
    bk = nc.dram_tensor("bk", [HD], F32, kind="ExternalInput")
    bv = nc.dram_tensor("bv", [HD], F32, kind="ExternalInput")
    woT = nc.dram_tensor("woT", [HD, E], BF16, kind="ExternalInput")
    bo2 = nc.dram_tensor("bo2", [E], F32, kind="ExternalInput")
    outT = nc.dram_tensor("outT", [E, S], F32, kind="ExternalOutput")

    with tile.TileContext(nc) as tc:
        mha_tile(
            tc,
            hs[:, :], wq[:, :], wk[:, :], wv[:, :],
            bq[:], bk[:], bv[:],
            woT[:, :], bo2[:], outT[:, :],
        )
    nc.compile()
    return nc


def make_core_inputs(inputs: dict) -> list[dict]:
    """Full inputs -> per-core input maps (core c: batch c//2, head-group c%2)."""
    bf16 = ml_dtypes.bfloat16
    hsf = np.ascontiguousarray(np.asarray(inputs["hidden_state"], dtype=np.float32))
    Wq = np.asarray(inputs["Wq"], dtype=np.float32)
    Wk = np.asarray(inputs["Wk"], dtype=np.float32)
    Wv = np.asarray(inputs["Wv"], dtype=np.float32)
    Wo = np.asarray(inputs["Wo"], dtype=np.float32)
    bq = np.asarray(inputs["bq"], dtype=np.float32)
    bk = np.asarray(inputs["bk"], dtype=np.float32)
    bv = np.asarray(inputs["bv"], dtype=np.float32)
    bo = np.asarray(inputs["bo"], dtype=np.float32)

    maps = []
    for c in range(8):
        b, g = c // 2, c % 2
        hsl = slice(NH * g, NH * (g + 1))
        fsl = slice(HD * g, HD * (g + 1))
        maps.append(
            {
                "hs": np.ascontiguousarray(hsf[b].astype(bf16)),
                "wq": np.ascontiguousarray(
                    Wq[hsl].transpose(1, 0, 2).reshape(E, HD).astype(bf16)
                ),
                "wk": np.ascontiguousarray(
                    Wk[hsl].transpose(1, 0, 2).reshape(E, HD).astype(bf16)
                ),
                "wv": np.ascontiguousarray(
                    Wv[hsl].transpose(1, 0, 2).reshape(E, HD).astype(bf16)
                ),
                "bq": np.ascontiguousarray(bq[hsl].reshape(HD)),
                "bk": np.ascontiguousarray(bk[hsl].reshape(HD)),
                "bv": np.ascontiguousarray(bv[hsl].reshape(HD)),
                "woT": np.ascontiguousarray(Wo[:, fsl].T.astype(bf16)),
                "bo2": np.ascontiguousarray(bo / 2.0),
            }
        )
    return maps


def combine_outputs(core_outs: list) -> np.ndarray:
    """Per-core outT partials -> full [B, E, S] output."""
    return np.stack(
        [core_outs[2 * b]["outT"] + core_outs[2 * b + 1]["outT"] for b in range(B)]
    ).astype(np.float32)


from concourse.bass_utils import run_bass_kernel_spmd

N_CORES = 8
_NC_CACHE = None


def _get_nc():
    global _NC_CACHE
    if _NC_CACHE is None:
        _NC_CACHE = build_nc()
    return _NC_CACHE


def kernel(**inputs) -> np.ndarray:
    """Full-input entry point: shard across 8 cores, run, unshard."""
    maps = make_core_inputs(inputs)
    nc = _get_nc()
    res = run_bass_kernel_spmd(nc, maps, core_ids=list(range(N_CORES)))
    outs = res.results
    return np.stack(
        [outs[2 * b]["outT"] + outs[2 * b + 1]["outT"] for b in range(B)]
    ).astype(np.float32)
